# revision 3
# baseline (speedup 1.0000x reference)
"""Trainium2 Bass kernel for nn_CanineAttention (block-diagonal local attention), v2.

Design (all-fp16 matmul operands, fp32 PSUM for projections):
  - Q/K projections -> qT,kT [h_out, t] (weights stationary, x^T moving N=512)
  - V projection -> v_aug [t, 12*(64+1)]: per-head 64 value dims plus a
    persistent ones-column: PV yields ctx AND the softmax denominator.
  - scores TRANSPOSED [k, q], fp16 PSUM, so exp batches per chunk:
    ONE activation op over 12 heads, fp16 out.
  - PV: ctx_aug[q, 65] = ex_h^T.T @ v_aug_h (ex stationary/FWL, v moving N=65)
  - normalize: ctx * (1/den) via stride-0 broadcast tensor_tensor
  - transpose ctx (6 PE transposes/chunk, fp16 PSUM) -> ctxT feeds O-proj
  - LayerNorm via bn_stats/bn_aggr + fused (h-mu)*rstd tensor_scalar;
    rsqrt via bit-trick + 3 Newton steps (DVE).
  - software pipeline: projections(b) emitted before attention(b-1) so PE
    interleaves GEMMs with the cross-engine attention chains.

Sharding: tokens (B*S=16384) split contiguously across 8 cores, data-parallel.
"""

import numpy as np
import ml_dtypes
from contextlib import ExitStack

import concourse.bass as bass
import concourse.tile as tile
from concourse import bacc, mybir
from concourse.bass import AP
from concourse.bass_utils import run_bass_kernel_spmd
from concourse.masks import make_identity

B, S, H, NH, W = 4, 4096, 768, 12, 128
HD = H // NH            # 64
C = S // W              # 32 chunks
NEG = -10000.0
EPS = 1e-12

NCORES = 8
TPC = B * S // NCORES   # 2048 tokens per core
CPC = TPC // W          # 16 chunks per core
BLK = 512               # tokens per processing block
NBLK = TPC // BLK       # 4 blocks
CPB = BLK // W          # 4 chunks per block
NG = H // 128           # 6 partition-chunks over H
NPAIR = NH // 2         # 6 head pairs

F32 = mybir.dt.float32
F16 = mybir.dt.float16
FP = mybir.ActivationFunctionType
OP = mybir.AluOpType
AX = mybir.AxisListType

MODE = "fp16"

# v_aug / ctx_aug column layout: head h ctx at 65h..65h+63, ones/den at 65h+64
VW = 65 * NH            # 780


def vcol(h):
    return 65 * h


# scores PSUM (fp32, 3 banks): even head 2g at col 128g, odd head 2g+1 at
# 768+128g. Banks (512 f32 cols): pairs always land in different banks.
SCW = 1536


def scol_ps(h):
    return 128 * (h // 2) + (768 if (h % 2) else 0)


scol_ex = scol_ps

# ctx_aug PSUM (fp32, 2 banks): heads 0-6 at 65h (bank0), 7-11 at 512+65(h-7).
def ccol(h):
    return 65 * h if h < 7 else 512 + 65 * (h - 7)

CW = 512 + 65 * 5       # 837


def _build(mode, use_mask, use_qbias, use_kbias, use_vbias, use_ln_affine, reps=1):
    nc = bacc.Bacc(
        "TRN2", target_bir_lowering=False, debug=False,
        enable_asserts=False, num_devices=NCORES,
    )
    wdt = F16

    names = []

    def dram_in(name, shape, dt):
        names.append(name)
        return nc.dram_tensor(name, shape, dt, kind="ExternalInput").ap()

    xt = dram_in("xt", [H, TPC], wdt)                  # x^T fp16
    xres = dram_in("xres", [TPC, H], F32)              # x + bo (residual)
    w_dram = {"wq": dram_in("wq", [H, H], wdt),
              "wk": dram_in("wk", [H, H], wdt),
              "wv": dram_in("wv", [H, H], wdt),
              "wo": dram_in("wo", [H, H], wdt)}
    bq = dram_in("bq", [128, NG], F32) if use_qbias else None
    bk = dram_in("bk", [128, NG], F32) if use_kbias else None
    bvb = dram_in("bvb", [128, VW], F32) if use_vbias else None   # aug layout
    gmb = dram_in("gmb", [128, H], F32) if use_ln_affine else None
    btb = dram_in("btb", [128, H], F32) if use_ln_affine else None
    mbias = dram_in("mbias", [CPC, W, W], F32) if use_mask else None  # [k,q]
    out = nc.dram_tensor("out", [TPC, H], F32, kind="ExternalOutput").ap()

    with tile.TileContext(nc) as tc, ExitStack() as ctx:
        const = ctx.enter_context(tc.tile_pool(name="const", bufs=1))
        xp = ctx.enter_context(tc.tile_pool(name="xp", bufs=2))
        qkv = ctx.enter_context(tc.tile_pool(name="qkv", bufs=2))
        attn = ctx.enter_context(tc.tile_pool(name="attn", bufs=2))
        ctxp = ctx.enter_context(tc.tile_pool(name="ctxp", bufs=2))
        outp = ctx.enter_context(tc.tile_pool(name="outp", bufs=2))
        # PSUM budget (8 banks): pproj 2x1 + psc 3 + pcx 2 + ppt 1 = 8
        pproj = ctx.enter_context(tc.tile_pool(name="pproj", bufs=2, space="PSUM"))
        psc = ctx.enter_context(tc.tile_pool(name="psc", bufs=1, space="PSUM"))
        pcx = ctx.enter_context(tc.tile_pool(name="pcx", bufs=1, space="PSUM"))
        ppt = ctx.enter_context(tc.tile_pool(name="ppt", bufs=1, space="PSUM"))

        ident = const.tile([128, 128], wdt, tag="ident")
        make_identity(nc, ident)

        w_sb = {}
        for wn, ap in w_dram.items():
            w_sb[wn] = []
            for g in range(NG):
                t = const.tile([128, H], wdt, tag=f"{wn}{g}")
                nc.sync.dma_start(t[:], ap[g * 128:(g + 1) * 128, :])
                w_sb[wn].append(t)

        bq_sb = bk_sb = bvb_sb = gmb_sb = btb_sb = None
        if use_qbias:
            bq_sb = const.tile([128, NG], F32, tag="bq")
            nc.sync.dma_start(bq_sb[:], bq)
        if use_kbias:
            bk_sb = const.tile([128, NG], F32, tag="bk")
            nc.sync.dma_start(bk_sb[:], bk)
        if use_vbias:
            bvb_sb = const.tile([128, VW], F32, tag="bvb")
            nc.sync.dma_start(bvb_sb[:], bvb)
        if use_ln_affine:
            gmb_sb = const.tile([128, H], F32, tag="gmb")
            nc.sync.dma_start(gmb_sb[:], gmb)
            btb_sb = const.tile([128, H], F32, tag="btb")
            nc.sync.dma_start(btb_sb[:], btb)

        # v_aug: 2 sets (block parity) x CPB tiles; ones columns written once,
        # per-block V copies never touch them.
        vsets = [[const.tile([128, VW], wdt, tag=f"vaug{p}_{tt}", name=f"vaug{p}_{tt}")
                  for tt in range(CPB)] for p in range(2)]
        for p in range(2):
            for tt in range(CPB):
                vt = vsets[p][tt]
                ones = AP(vt.tensor, vt[:].offset + 64, [vt[:].ap[0], [65, NH]])
                nc.vector.memset(ones, 1.0)

        def emit_xload(blk):
            t0 = blk * BLK
            xth = [xp.tile([128, BLK], wdt, tag=f"xth{g}", name=f"xth{g}")
                   for g in range(NG)]
            for g in range(NG):
                nc.sync.dma_start(xth[g][:], xt[g * 128:(g + 1) * 128, t0:t0 + BLK])
            return xth

        def emit_qk_group(xth, which, go):
            wn = "wq" if which == "q" else "wk"
            ps = pproj.tile([128, BLK], F32, tag="proj")
            for gi in range(NG):
                nc.tensor.matmul(
                    ps[:],
                    w_sb[wn][gi][:, go * 128:(go + 1) * 128],
                    xth[gi][:],
                    start=(gi == 0), stop=(gi == NG - 1),
                )
            sb = qkv.tile([128, BLK], wdt, tag=f"{which}T{go}", name=f"{which}T{go}")
            scl = 0.125 if which == "q" else 1.0
            has_b = use_qbias if which == "q" else use_kbias
            if has_b:
                bias = (bq_sb if which == "q" else bk_sb)[:, go:go + 1]
                nc.scalar.activation(sb[:], ps[:], FP.Identity, bias=bias, scale=scl)
            else:
                nc.scalar.activation(sb[:], ps[:], FP.Copy, scale=scl)
            return sb

        def emit_v_group(xth, vN, tt):
            vt = vN[tt]
            for nhalf in range(2):
                n0 = nhalf * 384
                ps = pproj.tile([128, 384], F32, tag="proj")
                for gi in range(NG):
                    nc.tensor.matmul(
                        ps[:],
                        xth[gi][:, tt * 128:(tt + 1) * 128],
                        w_sb["wv"][gi][:, n0:n0 + 384],
                        start=(gi == 0), stop=(gi == NG - 1),
                    )
                dst = AP(vt.tensor, vt[:].offset + (390 if nhalf else 0),
                         [vt[:].ap[0], [65, 6], [1, 64]])
                nc.vector.tensor_copy(dst, ps[:])
                if use_vbias:
                    bb = AP(bvb_sb.tensor, bvb_sb[:].offset + (390 if nhalf else 0),
                            [bvb_sb[:].ap[0], [65, 6], [1, 64]])
                    nc.vector.tensor_add(dst, dst, bb)

        def emit_scores(qT, kT, blk, cc):
            chunk_idx = blk * CPB + cc
            ts = slice(cc * 128, (cc + 1) * 128)
            ps_sc = psc.tile([128, SCW], F32, tag="sc", name=f"sc{cc}")
            for g in range(NPAIR):
                for h2 in range(2):
                    h = 2 * g + h2
                    p0 = h2 * 64
                    col = scol_ps(h)
                    nc.tensor.matmul(
                        ps_sc[:, col:col + 128],
                        kT[g][p0:p0 + 64, ts],
                        qT[g][p0:p0 + 64, ts],
                        start=True, stop=True,
                        tile_position=(p0, 0),
                        skip_group_check=(h2 == 1),
                    )
            if use_mask:
                mb = attn.tile([128, W], F32, tag="mb")
                nc.sync.dma_start(mb[:], mbias[chunk_idx])
                for h in range(NH):
                    col = scol_ps(h)
                    nc.vector.tensor_add(ps_sc[:, col:col + 128], ps_sc[:, col:col + 128], mb[:])
            ex = attn.tile([128, 1536], wdt, tag="ex", name=f"ex{cc}")
            nc.scalar.activation(ex[:], ps_sc[:], FP.Exp)
            return ex

        def emit_pv(ex, vN, cc):
            ctx_aug = pcx.tile([128, CW], F32, tag="cx")
            for h in range(NH):
                c0 = scol_ex(h)
                d0 = ccol(h)
                nc.tensor.matmul(
                    ctx_aug[:, d0:d0 + 65],
                    ex[:, c0:c0 + 128],
                    vN[cc][:, vcol(h):vcol(h) + 65],
                    start=True, stop=True,
                )
            rec = attn.tile([128, NH], F32, tag="rec")
            s7 = AP(ctx_aug.tensor, ctx_aug[:].offset + 64, [ctx_aug[:].ap[0], [65, 7]])
            nc.vector.reciprocal(rec[:, 0:7], s7)
            s5 = AP(ctx_aug.tensor, ctx_aug[:].offset + 512 + 64, [ctx_aug[:].ap[0], [65, 5]])
            nc.vector.reciprocal(rec[:, 7:12], s5)
            cx = ctxp.tile([128, H], wdt, tag="cxn")
            i7 = AP(ctx_aug.tensor, ctx_aug[:].offset, [ctx_aug[:].ap[0], [65, 7], [1, 64]])
            r7 = AP(rec.tensor, rec[:].offset, [rec[:].ap[0], [1, 7], [0, 64]])
            nc.vector.tensor_mul(cx[:, 0:448], i7, r7)
            i5 = AP(ctx_aug.tensor, ctx_aug[:].offset + 512, [ctx_aug[:].ap[0], [65, 5], [1, 64]])
            r5 = AP(rec.tensor, rec[:].offset + 7, [rec[:].ap[0], [1, 5], [0, 64]])
            nc.vector.tensor_mul(cx[:, 448:768], i5, r5)
            pt = ppt.tile([128, H], wdt, tag="pt")
            for g in range(NG):
                nc.tensor.matmul(
                    pt[:, g * 128:(g + 1) * 128],
                    cx[:, g * 128:(g + 1) * 128],
                    ident[:], is_transpose=True,
                    skip_group_check=(g > 0),
                )
            ct = ctxp.tile([128, H], wdt, tag=f"cxT{cc}", name=f"cxT{cc}")
            nc.vector.tensor_copy(ct[:], pt[:])
            return ct

        def emit_oproj_tile(cxT, blk, tt):
            t0 = blk * BLK
            r0 = t0 + tt * 128
            xr = outp.tile([128, H], F32, tag="xr")
            nc.sync.dma_start(xr[:], xres[r0:r0 + 128, :])
            hsb = outp.tile([128, H], F32, tag="hsb")
            for nhalf in range(2):
                n0 = nhalf * 384
                ps = pproj.tile([128, 384], F32, tag="proj")
                for gi in range(NG):
                    nc.tensor.matmul(
                        ps[:],
                        cxT[tt][:, gi * 128:(gi + 1) * 128],
                        w_sb["wo"][gi][:, n0:n0 + 384],
                        start=(gi == 0), stop=(gi == NG - 1),
                    )
                nc.vector.tensor_add(hsb[:, n0:n0 + 384], ps[:], xr[:, n0:n0 + 384])

            st1 = outp.tile([128, 12], F32, tag="st1")
            nc.vector.bn_stats(st1[:, 0:6], hsb[:, 0:384])
            nc.vector.bn_stats(st1[:, 6:12], hsb[:, 384:768])
            st2 = outp.tile([128, 2], F32, tag="st2")
            nc.vector.bn_aggr(st2[:], st1[:])
            mu = st2[:, 0:1]
            var1 = outp.tile([128, 1], F32, tag="var1")
            nc.vector.tensor_scalar(var1[:], st2[:, 1:2], 1.0, EPS, op0=OP.mult, op1=OP.add)
            rstd = outp.tile([128, 1], F32, tag="rstd")
            t1 = outp.tile([128, 1], F32, tag="t1n")
            ri = rstd[:].bitcast(mybir.dt.int32)
            nc.vector.tensor_scalar(
                ri, var1[:].bitcast(mybir.dt.int32), 1, None,
                op0=OP.logical_shift_right,
            )
            nc.vector.tensor_scalar(ri, ri, -1, 0x5F3759DF, op0=OP.mult, op1=OP.add)
            for _ in range(3):
                nc.vector.tensor_mul(t1[:], rstd[:], rstd[:])
                nc.vector.tensor_mul(t1[:], t1[:], var1[:])
                nc.vector.tensor_scalar(t1[:], t1[:], -0.5, 1.5, op0=OP.mult, op1=OP.add)
                nc.vector.tensor_mul(rstd[:], rstd[:], t1[:])
            ot = outp.tile([128, H], F32, tag="ot")
            nc.vector.tensor_scalar(ot[:], hsb[:], mu, rstd[:], op0=OP.subtract, op1=OP.mult)
            if use_ln_affine:
                nc.vector.tensor_mul(ot[:], ot[:], gmb_sb[:])
                nc.vector.tensor_add(ot[:], ot[:], btb_sb[:])
            nc.sync.dma_start(out[r0:r0 + 128, :], ot[:])

        import contextlib
        rep_cm = tc.For_i(0, reps, 1) if reps > 1 else contextlib.nullcontext()
        with rep_cm:
            # block 0 projections (nothing to overlap with)
            xth = emit_xload(0)
            qT = [emit_qk_group(xth, "q", go) for go in range(NG)]
            kT = [emit_qk_group(xth, "k", go) for go in range(NG)]
            vN = vsets[0]
            for tt in range(CPB):
                emit_v_group(xth, vN, tt)
            prev = (qT, kT, vN, 0)

            for blk in range(1, NBLK):
                pqT, pkT, pvN, pblk = prev
                xth = emit_xload(blk)
                # interleave: attention(blk-1) pieces between projection(blk)
                # groups so the in-order PE queue always has ready GEMMs ahead
                # of exp-blocked score matmuls.
                ex0 = emit_scores(pqT, pkT, pblk, 0)
                qT = [emit_qk_group(xth, "q", go) for go in range(3)]
                ex1 = emit_scores(pqT, pkT, pblk, 1)
                ct0 = emit_pv(ex0, pvN, 0)
                qT += [emit_qk_group(xth, "q", go) for go in range(3, NG)]
                ex2 = emit_scores(pqT, pkT, pblk, 2)
                ct1 = emit_pv(ex1, pvN, 1)
                kT = [emit_qk_group(xth, "k", go) for go in range(3)]
                ex3 = emit_scores(pqT, pkT, pblk, 3)
                ct2 = emit_pv(ex2, pvN, 2)
                kT += [emit_qk_group(xth, "k", go) for go in range(3, NG)]
                ct3 = emit_pv(ex3, pvN, 3)
                cxT = [ct0, ct1, ct2, ct3]
                vN = vsets[blk % 2]
                emit_v_group(xth, vN, 0)
                emit_v_group(xth, vN, 1)
                emit_oproj_tile(cxT, pblk, 0)
                emit_v_group(xth, vN, 2)
                emit_oproj_tile(cxT, pblk, 1)
                emit_v_group(xth, vN, 3)
                emit_oproj_tile(cxT, pblk, 2)
                emit_oproj_tile(cxT, pblk, 3)
                prev = (qT, kT, vN, blk)

            # tail: attention + output for the last block
            pqT, pkT, pvN, pblk = prev
            ex0 = emit_scores(pqT, pkT, pblk, 0)
            ex1 = emit_scores(pqT, pkT, pblk, 1)
            ct0 = emit_pv(ex0, pvN, 0)
            ex2 = emit_scores(pqT, pkT, pblk, 2)
            ct1 = emit_pv(ex1, pvN, 1)
            emit_oproj_tile([ct0], pblk, 0)
            ex3 = emit_scores(pqT, pkT, pblk, 3)
            ct2 = emit_pv(ex2, pvN, 2)
            emit_oproj_tile([None, ct1], pblk, 1)
            ct3 = emit_pv(ex3, pvN, 3)
            emit_oproj_tile([None, None, ct2], pblk, 2)
            emit_oproj_tile([None, None, None, ct3], pblk, 3)

    nc.compile()
    return nc, names


_CACHE = {}


def _get_program(mode, use_mask, use_qbias, use_kbias, use_vbias, use_ln_affine, reps=1):
    key = (mode, use_mask, use_qbias, use_kbias, use_vbias, use_ln_affine, reps)
    if key not in _CACHE:
        _CACHE[key] = _build(*key[:-1], reps=reps)
    return _CACHE[key]


def _prep_inputs(inputs, mode=None):
    hs = np.ascontiguousarray(np.asarray(inputs["hidden_states"], dtype=np.float32))
    mask = np.asarray(inputs["attention_mask"], dtype=np.float32)
    Wq = np.asarray(inputs["Wq"], np.float32); bq = np.asarray(inputs["bq"], np.float32)
    Wk = np.asarray(inputs["Wk"], np.float32); bk = np.asarray(inputs["bk"], np.float32)
    Wv = np.asarray(inputs["Wv"], np.float32); bv = np.asarray(inputs["bv"], np.float32)
    Wo = np.asarray(inputs["Wo"], np.float32); bo = np.asarray(inputs["bo"], np.float32)
    gm = np.asarray(inputs["ln_gamma"], np.float32)
    bt = np.asarray(inputs["ln_beta"], np.float32)

    npdt = np.float16
    use_mask = not np.all(mask == 1.0)
    use_qbias = bool(np.any(bq)); use_kbias = bool(np.any(bk))
    use_vbias = bool(np.any(bv))
    use_ln_affine = bool(np.any(gm != 1.0) or np.any(bt))

    x = hs.reshape(B * S, H)
    xres_full = x + bo[None, :] if np.any(bo) else x

    wq, wk, wv, wo = (np.ascontiguousarray(w.astype(npdt)) for w in (Wq, Wk, Wv, Wo))

    if use_mask:
        m4 = mask.reshape(B, C, W, C, W)
        idx = np.arange(C)
        mblk = m4[:, idx, :, idx, :]                 # [C,B,W,W]
        mblk = np.transpose(mblk, (1, 0, 3, 2))      # [B,C,W,W] transposed [k,q]
        bias_blocks = ((1.0 - mblk) * NEG).astype(np.float32).reshape(B * C, W, W)
    if use_vbias:
        bvb_aug = np.zeros((128, VW), np.float32)
        for h in range(NH):
            bvb_aug[:, vcol(h):vcol(h) + 64] = bv[64 * h:64 * h + 64][None, :]

    in_maps = []
    for c in range(NCORES):
        sl = x[c * TPC:(c + 1) * TPC]
        m = {}
        m["xt"] = np.ascontiguousarray(sl.astype(npdt).T)
        m["xres"] = np.ascontiguousarray(xres_full[c * TPC:(c + 1) * TPC])
        m["wq"] = wq; m["wk"] = wk; m["wv"] = wv; m["wo"] = wo
        if use_qbias:
            m["bq"] = np.ascontiguousarray((bq / 8.0).reshape(NG, 128).T)
        if use_kbias:
            m["bk"] = np.ascontiguousarray(bk.reshape(NG, 128).T)
        if use_vbias:
            m["bvb"] = bvb_aug
        if use_ln_affine:
            m["gmb"] = np.ascontiguousarray(np.broadcast_to(gm, (128, H)))
            m["btb"] = np.ascontiguousarray(np.broadcast_to(bt, (128, H)))
        if use_mask:
            m["mbias"] = np.ascontiguousarray(bias_blocks[c * CPC:(c + 1) * CPC])
        in_maps.append(m)

    flags = (use_mask, use_qbias, use_kbias, use_vbias, use_ln_affine)
    return in_maps, flags


def run(inputs, mode=None, trace=False, reps=1):
    mode = mode or MODE
    in_maps, flags = _prep_inputs(inputs, mode)
    nc, names = _get_program(mode, *flags, reps=reps)
    in_maps = [{k: v for k, v in m.items() if k in names} for m in in_maps]
    res = run_bass_kernel_spmd(nc, in_maps, list(range(NCORES)), trace=trace)
    outs = [res.results[c]["out"] for c in range(NCORES)]
    full = np.concatenate(outs, axis=0).reshape(B, S, H).astype(np.float32)
    return full, res


def kernel(**inputs):
    out, _ = run(inputs)
    return out
